# revision 20
# baseline (speedup 1.0000x reference)
"""Trainium2 Bass kernel for nn_Meta_Graph1_40114994545303 (gnn_message_passing).

Math: the reference returns only the global-node row of the GCN output.
With mask = (attribute_label > 0), star adjacency means
    out[s, :] = tanh( (sum_a mask[s,a] * attribute_feat[s,a,:]) @ W + b )
and x never reaches the output (adj[A, A] = 0).

Data-parallel over batch: 32 samples per core on 8 cores, W replicated (f16).
Per core:
  stage 1: masked sum over attributes. The host packs only the masked rows
    (mask=1) of attribute_feat contiguously (zero-padded to K1P*128 rows),
    so the feat stream carries ~half the bytes; a host-built block-select
    matrix (ones at [row -> sample]) is the stationary operand and the
    packed feat rows stream through. Four serial 512-col matmuls per
    k-chunk into four psum banks at partitions 0:32 (sample-aligned).
  transpose: psum -> msc_nat [32, 2048] f16 (4 DVE copies, same partitions),
    then ONE SBUF->SBUF DMA-transpose to msT [128, 16, 32] (d on partitions).
  stage 2: [32, 2048] @ W as 16 K-chunk matmuls, msT chunks stationary and
    W streamed 512-col col-tiled x4; bias folded in as a rank-1 matmul of
    ones x b into the same PSUM accumulation; tanh on scalar engine.

Fallback `dp8` (full unpacked feat, DVE 32x32 transposes) is compiled only
if an input exceeds the packed-row capacity.
"""

import os
from contextlib import ExitStack

import numpy as np

import concourse.bacc as bacc
import concourse.mybir as mybir

B, A, D = 256, 32, 2048
NCORES = 8
S = B // NCORES  # 32 samples per core
P = 128
KC1 = (S * A) // P  # 8 k-chunks of unpacked feat
K1P = 5  # packed-feat k-chunks (640 rows; P(Binom(1024,.5) > 640) ~ 6e-16)
KC2 = D // P  # 16 k-chunks in stage 2
NT = D // 512  # 4 psum-bank column tiles
F32 = mybir.dt.float32
F16 = mybir.dt.float16


def build_nc_packed(out16: bool = False):
    WCH = [4, 4, 4, 2, 2]  # W transfer sizes (k2-chunks); short tail
    WST = [0, 4, 8, 12, 14]
    FCH = [3, 2]  # packed-feat transfer sizes (k1-chunks)
    FST = [0, 3]
    NF, NW = len(FCH), len(WCH)
    nc = bacc.Bacc("TRN2", target_bir_lowering=False, debug=False)

    feat = nc.dram_tensor("feat", [K1P * P, D], F16, kind="ExternalInput")
    mbdt = nc.dram_tensor("mbdt", [P, K1P * S], F16, kind="ExternalInput")
    w = nc.dram_tensor("w", [D, D], F16, kind="ExternalInput")
    bias = nc.dram_tensor("bias", [1, D], F16, kind="ExternalInput")
    onesd = nc.dram_tensor("ones", [1, S], F16, kind="ExternalInput")
    if out16:
        # group-blocked: outb[g*S+s, u] = out[s, g*512+u]; host un-blocks
        out = nc.dram_tensor("outb", [P, 512], F16, kind="ExternalOutput")
    else:
        out = nc.dram_tensor("out", [S, D], F32, kind="ExternalOutput")

    with ExitStack() as ctx:
        feat_sb = ctx.enter_context(nc.sbuf_tensor([P, K1P, D], F16))
        w_sb = ctx.enter_context(nc.sbuf_tensor([P, KC2, D], F16))
        mbdt_sb = ctx.enter_context(nc.sbuf_tensor([P, K1P, S], F16))
        bias_sb = ctx.enter_context(nc.sbuf_tensor([1, D], F16))
        ones_sb = ctx.enter_context(nc.sbuf_tensor([1, S], F16))
        msc_sb = ctx.enter_context(nc.sbuf_tensor([S, D], F16))
        msT_sb = ctx.enter_context(nc.sbuf_tensor([P, KC2, S], F16))
        out_sb = ctx.enter_context(
            nc.sbuf_tensor([P, 512], F16 if out16 else F32)
        )
        pm_banks = [
            ctx.enter_context(nc.psum_tensor(f"pm{n}", [P, 512], F32))
            for n in range(NT)
        ]
        po_bank = ctx.enter_context(nc.psum_tensor([P, 512], F32))
        fsems = [ctx.enter_context(nc.semaphore(f"fs{g}")) for g in range(NF)]
        wsems = [ctx.enter_context(nc.semaphore(f"ws{g}")) for g in range(NW)]
        csem = ctx.enter_context(nc.semaphore("csem"))
        osem = ctx.enter_context(nc.semaphore("osem"))
        s1_sem = ctx.enter_context(nc.semaphore("s1_sem"))
        cp_sem = ctx.enter_context(nc.semaphore("cp_sem"))
        tr_sem = ctx.enter_context(nc.semaphore("tr_sem"))
        s2_sem = ctx.enter_context(nc.semaphore("s2_sem"))
        act_sem = ctx.enter_context(nc.semaphore("act_sem"))
        osem2 = ctx.enter_context(nc.semaphore("osem2"))
        block = ctx.enter_context(nc.Block(no_gpsimd_drain=True))

        @block.sync
        def _(sync):
            for g in range(NF):
                st, ln = FST[g], FCH[g]
                sync.dma_start(
                    feat_sb[:, st : st + ln, :],
                    feat[st * P : (st + ln) * P, :].rearrange(
                        "(c p) d -> p c d", p=P
                    ),
                ).then_inc(fsems[g], 16)
            for g in range(NW):
                st, ln = WST[g], WCH[g]
                sync.dma_start(
                    w_sb[:, st : st + ln, :],
                    w[st * P : (st + ln) * P, :].rearrange("(c p) d -> p c d", p=P),
                ).then_inc(wsems[g], 16)
            sync.wait_ge(act_sem, 1)
            if out16:
                sync.dma_start(out[:, :], out_sb[:, :]).then_inc(osem2, 16)
                sync.wait_ge(osem2, 16)
            else:
                for n in (0, 2):
                    sync.dma_start(
                        out[:, n * 512 : (n + 1) * 512],
                        out_sb[n * S : (n + 1) * S, :],
                    ).then_inc(osem2, 16)
                sync.wait_ge(osem2, 32)

        @block.scalar
        def _(scalar):
            scalar.dma_start(
                mbdt_sb[:], mbdt[:].rearrange("p (k j) -> p k j", k=K1P)
            ).then_inc(csem, 16)
            scalar.dma_start(bias_sb[:], bias[:]).then_inc(csem, 16)
            scalar.dma_start(ones_sb[:], onesd[:]).then_inc(csem, 16)
            # the one SBUF->SBUF transpose: msc_nat [32, 2048] -> [128, 16, 32]
            scalar.wait_ge(cp_sem, NT)
            scalar.dma_start(msT_sb[:, :, :], msc_sb[:, :], transpose=True).then_inc(
                tr_sem, 16
            )
            scalar.wait_ge(s2_sem, 1)
            nc.scalar.activation(
                out_sb[:], po_bank[:], mybir.ActivationFunctionType.Tanh
            ).then_inc(act_sem, 1)
            if not out16:
                scalar.wait_ge(act_sem, 1)
                for n in (1, 3):
                    scalar.dma_start(
                        out[:, n * 512 : (n + 1) * 512],
                        out_sb[n * S : (n + 1) * S, :],
                    ).then_inc(osem, 16)
                scalar.wait_ge(osem, 32)

        @block.vector
        def _(vector):
            vector.wait_ge(s1_sem, 1)
            last = None
            for n in range(NT):
                last = nc.vector.tensor_copy(
                    msc_sb[:, n * 512 : (n + 1) * 512], pm_banks[n][0:S, :]
                )
            last.then_inc(cp_sem, NT)

        @block.tensor
        def _(tensor):
            tensor.wait_ge(csem, 48)
            for n in range(NT):
                nc.tensor.matmul(
                    po_bank[n * S : (n + 1) * S, :],
                    ones_sb[:],
                    bias_sb[:, n * 512 : (n + 1) * 512],
                    start=True,
                    stop=False,
                    tile_position=(0, n * S),
                    skip_group_check=True,
                )
            last = None
            for g in range(NF):
                tensor.wait_ge(fsems[g], 16)
                for c in range(FCH[g]):
                    k = FST[g] + c
                    for n in range(NT):
                        last = nc.tensor.matmul(
                            pm_banks[n][0:S, :],
                            mbdt_sb[:, k, :],
                            feat_sb[:, k, n * 512 : (n + 1) * 512],
                            start=(k == 0),
                            stop=(k == K1P - 1),
                            skip_group_check=True,
                        )
            last.then_inc(s1_sem, 1)
            tensor.wait_ge(tr_sem, 16)
            lastb = None
            for g in range(NW):
                tensor.wait_ge(wsems[g], 16)
                for c in range(WCH[g]):
                    k2 = WST[g] + c
                    for n in range(NT):
                        lastb = nc.tensor.matmul(
                            po_bank[n * S : (n + 1) * S, :],
                            msT_sb[:, k2, :],
                            w_sb[:, k2, n * 512 : (n + 1) * 512],
                            start=False,
                            stop=(k2 == KC2 - 1),
                            tile_position=(0, n * S),
                            skip_group_check=True,
                        )
            lastb.then_inc(s2_sem, 1)

    nc.compile()
    return nc


def build_nc_packed5():
    """packed4 + whole stream on the SWDGE ring (feat then W sequential,
    ~362 GB/s vs HWDGE ~325), 1 MB W transfers (inter-arrival idle < HAM
    window), HAM warm-up dummies in the PE-idle windows, fp16 single out."""
    FCH = [3, 2]
    FST = [0, 3]
    NF = len(FCH)
    WCH = [2] * 8
    WST = [2 * g for g in range(8)]
    NW = len(WCH)
    nc = bacc.Bacc("TRN2", target_bir_lowering=False, debug=False)

    feat = nc.dram_tensor("feat", [K1P * P, D], F16, kind="ExternalInput")
    mbdt = nc.dram_tensor("mbdt", [P, K1P * S], F16, kind="ExternalInput")
    w = nc.dram_tensor("w", [D, D], F16, kind="ExternalInput")
    bias = nc.dram_tensor("bias", [1, D], F16, kind="ExternalInput")
    onesd = nc.dram_tensor("ones", [1, S], F16, kind="ExternalInput")
    outb = nc.dram_tensor("outb", [P, 512], F16, kind="ExternalOutput")

    with ExitStack() as ctx:
        feat_sb = ctx.enter_context(nc.sbuf_tensor([P, K1P, D], F16))
        w_sb = ctx.enter_context(nc.sbuf_tensor([P, KC2, D], F16))
        mbdt_sb = ctx.enter_context(nc.sbuf_tensor([P, K1P, S], F16))
        bias_sb = ctx.enter_context(nc.sbuf_tensor([1, D], F16))
        ones_sb = ctx.enter_context(nc.sbuf_tensor([1, S], F16))
        msc_sb = ctx.enter_context(nc.sbuf_tensor([S, D], F16))
        msT_sb = ctx.enter_context(nc.sbuf_tensor([P, KC2, S], F16))
        out_sb = ctx.enter_context(nc.sbuf_tensor([P, 512], F16))
        pm_banks = [
            ctx.enter_context(nc.psum_tensor(f"pm{n}", [P, 512], F32))
            for n in range(NT)
        ]
        po_bank = ctx.enter_context(nc.psum_tensor([P, 512], F32))
        pw_bank = ctx.enter_context(nc.psum_tensor([P, S], F32))
        fsems = [ctx.enter_context(nc.semaphore(f"fs{g}")) for g in range(NF)]
        wsems = [ctx.enter_context(nc.semaphore(f"ws{g}")) for g in range(NW)]
        csem = ctx.enter_context(nc.semaphore("csem"))
        s1_sem = ctx.enter_context(nc.semaphore("s1_sem"))
        cp_sem = ctx.enter_context(nc.semaphore("cp_sem"))
        tr_sem = ctx.enter_context(nc.semaphore("tr_sem"))
        s2_sem = ctx.enter_context(nc.semaphore("s2_sem"))
        act_sem = ctx.enter_context(nc.semaphore("act_sem"))
        osem = ctx.enter_context(nc.semaphore("osem"))
        block = ctx.enter_context(nc.Block(no_gpsimd_drain=True))

        @block.gpsimd
        def _(gp):
            # single SWDGE ring, FIFO: feat fully precedes W
            for g in range(NF):
                st, ln = FST[g], FCH[g]
                gp.dma_start(
                    feat_sb[:, st : st + ln, :],
                    feat[st * P : (st + ln) * P, :].rearrange(
                        "(c p) d -> p c d", p=P
                    ),
                ).then_inc(fsems[g], 16)
            for g in range(NW):
                st, ln = WST[g], WCH[g]
                gp.dma_start(
                    w_sb[:, st : st + ln, :],
                    w[st * P : (st + ln) * P, :].rearrange("(c p) d -> p c d", p=P),
                ).then_inc(wsems[g], 16)

        @block.sync
        def _(sync):
            sync.wait_ge(act_sem, 1)
            sync.dma_start(outb[:, :], out_sb[:, :]).then_inc(osem, 16)
            sync.wait_ge(osem, 16)

        @block.scalar
        def _(scalar):
            scalar.dma_start(
                mbdt_sb[:], mbdt[:].rearrange("p (k j) -> p k j", k=K1P)
            ).then_inc(csem, 16)
            scalar.dma_start(bias_sb[:], bias[:]).then_inc(csem, 16)
            scalar.dma_start(ones_sb[:], onesd[:]).then_inc(csem, 16)
            scalar.wait_ge(cp_sem, NT)
            scalar.dma_start(msT_sb[:, :, :], msc_sb[:, :], transpose=True).then_inc(
                tr_sem, 16
            )
            scalar.wait_ge(s2_sem, 1)
            nc.scalar.activation(
                out_sb[:], po_bank[:], mybir.ActivationFunctionType.Tanh
            ).then_inc(act_sem, 1)

        @block.vector
        def _(vector):
            vector.wait_ge(s1_sem, 1)
            last = None
            for n in range(NT):
                last = nc.vector.tensor_copy(
                    msc_sb[:, n * 512 : (n + 1) * 512], pm_banks[n][0:S, :]
                )
            last.then_inc(cp_sem, NT)

        def dummies(count):
            for _ in range(count):
                nc.tensor.matmul(
                    pw_bank[0:S, 0:S],
                    mbdt_sb[:, 0, :],
                    mbdt_sb[:, 0, :],
                    start=True,
                    stop=True,
                    skip_group_check=True,
                )

        @block.tensor
        def _(tensor):
            tensor.wait_ge(csem, 48)
            for n in range(NT):
                nc.tensor.matmul(
                    po_bank[n * S : (n + 1) * S, :],
                    ones_sb[:],
                    bias_sb[:, n * 512 : (n + 1) * 512],
                    start=True,
                    stop=False,
                    tile_position=(0, n * S),
                    skip_group_check=True,
                )
            dummies(44)  # warm the PE through the pre-feat idle (~4 us)
            last = None
            for g in range(NF):
                tensor.wait_ge(fsems[g], 16)
                for c in range(FCH[g]):
                    k = FST[g] + c
                    for n in range(NT):
                        last = nc.tensor.matmul(
                            pm_banks[n][0:S, :],
                            mbdt_sb[:, k, :],
                            feat_sb[:, k, n * 512 : (n + 1) * 512],
                            start=(k == 0),
                            stop=(k == K1P - 1),
                            skip_group_check=True,
                        )
            last.then_inc(s1_sem, 1)
            dummies(20)  # stay warm across the copy+transpose window (~2 us)
            tensor.wait_ge(tr_sem, 16)
            lastb = None
            for g in range(NW):
                tensor.wait_ge(wsems[g], 16)
                for c in range(WCH[g]):
                    k2 = WST[g] + c
                    for n in range(NT):
                        lastb = nc.tensor.matmul(
                            po_bank[n * S : (n + 1) * S, :],
                            msT_sb[:, k2, :],
                            w_sb[:, k2, n * 512 : (n + 1) * 512],
                            start=False,
                            stop=(k2 == KC2 - 1),
                            tile_position=(0, n * S),
                            skip_group_check=True,
                        )
            lastb.then_inc(s2_sem, 1)

    nc.compile()
    return nc


def build_nc_packed3():
    """packed + W on the SWDGE (gpsimd) ring concurrent with feat on sync
    (SWDGE measured ~362 GB/s vs HWDGE ~325), single contiguous fp16 out
    transfer in group-blocked layout (host un-blocks), 1-chunk W tail."""
    FCH = [3, 2]
    FST = [0, 3]
    NF = len(FCH)
    WCH = [4, 4, 4, 3, 1]
    WST = [0, 4, 8, 12, 15]
    NW = len(WCH)
    nc = bacc.Bacc("TRN2", target_bir_lowering=False, debug=False)

    feat = nc.dram_tensor("feat", [K1P * P, D], F16, kind="ExternalInput")
    mbdt = nc.dram_tensor("mbdt", [P, K1P * S], F16, kind="ExternalInput")
    w = nc.dram_tensor("w", [D, D], F16, kind="ExternalInput")
    bias = nc.dram_tensor("bias", [1, D], F16, kind="ExternalInput")
    onesd = nc.dram_tensor("ones", [1, S], F16, kind="ExternalInput")
    # group-blocked: outb[g*S+s, u] = out_full[s, g*512+u], host reshapes
    outb = nc.dram_tensor("outb", [P, 512], F16, kind="ExternalOutput")

    with ExitStack() as ctx:
        feat_sb = ctx.enter_context(nc.sbuf_tensor([P, K1P, D], F16))
        w_sb = ctx.enter_context(nc.sbuf_tensor([P, KC2, D], F16))
        mbdt_sb = ctx.enter_context(nc.sbuf_tensor([P, K1P, S], F16))
        bias_sb = ctx.enter_context(nc.sbuf_tensor([1, D], F16))
        ones_sb = ctx.enter_context(nc.sbuf_tensor([1, S], F16))
        msc_sb = ctx.enter_context(nc.sbuf_tensor([S, D], F16))
        msT_sb = ctx.enter_context(nc.sbuf_tensor([P, KC2, S], F16))
        out_sb = ctx.enter_context(nc.sbuf_tensor([P, 512], F16))
        pm_banks = [
            ctx.enter_context(nc.psum_tensor(f"pm{n}", [P, 512], F32))
            for n in range(NT)
        ]
        po_bank = ctx.enter_context(nc.psum_tensor([P, 512], F32))
        fsems = [ctx.enter_context(nc.semaphore(f"fs{g}")) for g in range(NF)]
        wsems = [ctx.enter_context(nc.semaphore(f"ws{g}")) for g in range(NW)]
        csem = ctx.enter_context(nc.semaphore("csem"))
        s1_sem = ctx.enter_context(nc.semaphore("s1_sem"))
        cp_sem = ctx.enter_context(nc.semaphore("cp_sem"))
        tr_sem = ctx.enter_context(nc.semaphore("tr_sem"))
        s2_sem = ctx.enter_context(nc.semaphore("s2_sem"))
        act_sem = ctx.enter_context(nc.semaphore("act_sem"))
        osem = ctx.enter_context(nc.semaphore("osem"))
        block = ctx.enter_context(nc.Block(no_gpsimd_drain=True))

        @block.sync
        def _(sync):
            for g in range(NF):
                st, ln = FST[g], FCH[g]
                sync.dma_start(
                    feat_sb[:, st : st + ln, :],
                    feat[st * P : (st + ln) * P, :].rearrange(
                        "(c p) d -> p c d", p=P
                    ),
                ).then_inc(fsems[g], 16)
            sync.wait_ge(act_sem, 1)
            sync.dma_start(outb[:, :], out_sb[:, :]).then_inc(osem, 16)
            sync.wait_ge(osem, 16)

        @block.gpsimd
        def _(gp):
            for g in range(NW):
                st, ln = WST[g], WCH[g]
                gp.dma_start(
                    w_sb[:, st : st + ln, :],
                    w[st * P : (st + ln) * P, :].rearrange("(c p) d -> p c d", p=P),
                ).then_inc(wsems[g], 16)

        @block.scalar
        def _(scalar):
            scalar.dma_start(
                mbdt_sb[:], mbdt[:].rearrange("p (k j) -> p k j", k=K1P)
            ).then_inc(csem, 16)
            scalar.dma_start(bias_sb[:], bias[:]).then_inc(csem, 16)
            scalar.dma_start(ones_sb[:], onesd[:]).then_inc(csem, 16)
            scalar.wait_ge(cp_sem, NT)
            scalar.dma_start(msT_sb[:, :, :], msc_sb[:, :], transpose=True).then_inc(
                tr_sem, 16
            )
            scalar.wait_ge(s2_sem, 1)
            nc.scalar.activation(
                out_sb[:], po_bank[:], mybir.ActivationFunctionType.Tanh
            ).then_inc(act_sem, 1)

        @block.vector
        def _(vector):
            vector.wait_ge(s1_sem, 1)
            last = None
            for n in range(NT):
                last = nc.vector.tensor_copy(
                    msc_sb[:, n * 512 : (n + 1) * 512], pm_banks[n][0:S, :]
                )
            last.then_inc(cp_sem, NT)

        @block.tensor
        def _(tensor):
            tensor.wait_ge(csem, 48)
            for n in range(NT):
                nc.tensor.matmul(
                    po_bank[n * S : (n + 1) * S, :],
                    ones_sb[:],
                    bias_sb[:, n * 512 : (n + 1) * 512],
                    start=True,
                    stop=False,
                    tile_position=(0, n * S),
                    skip_group_check=True,
                )
            last = None
            for g in range(NF):
                tensor.wait_ge(fsems[g], 16)
                for c in range(FCH[g]):
                    k = FST[g] + c
                    for n in range(NT):
                        last = nc.tensor.matmul(
                            pm_banks[n][0:S, :],
                            mbdt_sb[:, k, :],
                            feat_sb[:, k, n * 512 : (n + 1) * 512],
                            start=(k == 0),
                            stop=(k == K1P - 1),
                            skip_group_check=True,
                        )
            last.then_inc(s1_sem, 1)
            tensor.wait_ge(tr_sem, 16)
            lastb = None
            for g in range(NW):
                tensor.wait_ge(wsems[g], 16)
                for c in range(WCH[g]):
                    k2 = WST[g] + c
                    for n in range(NT):
                        lastb = nc.tensor.matmul(
                            po_bank[n * S : (n + 1) * S, :],
                            msT_sb[:, k2, :],
                            w_sb[:, k2, n * 512 : (n + 1) * 512],
                            start=False,
                            stop=(k2 == KC2 - 1),
                            tile_position=(0, n * S),
                            skip_group_check=True,
                        )
            lastb.then_inc(s2_sem, 1)

    nc.compile()
    return nc


def build_nc_packed2():
    """packed + W stream split across both HWDGE rings (sync: k2 0-7 after
    feat; scalar: k2 8-15 after the small consts) for higher aggregate DMA
    throughput. Stage-2 consumes the scalar half first (it lands earlier)."""
    FCH = [3, 2]
    FST = [0, 3]
    NF = len(FCH)
    WA = [(0, 4), (4, 4)]  # sync-ring W transfers (start, len)
    WB = [(8, 4), (12, 2), (14, 2)]  # scalar-ring W transfers
    nc = bacc.Bacc("TRN2", target_bir_lowering=False, debug=False)

    feat = nc.dram_tensor("feat", [K1P * P, D], F16, kind="ExternalInput")
    mbdt = nc.dram_tensor("mbdt", [P, K1P * S], F16, kind="ExternalInput")
    w = nc.dram_tensor("w", [D, D], F16, kind="ExternalInput")
    bias = nc.dram_tensor("bias", [1, D], F16, kind="ExternalInput")
    onesd = nc.dram_tensor("ones", [1, S], F16, kind="ExternalInput")
    out = nc.dram_tensor("out", [S, D], F32, kind="ExternalOutput")

    with ExitStack() as ctx:
        feat_sb = ctx.enter_context(nc.sbuf_tensor([P, K1P, D], F16))
        w_sb = ctx.enter_context(nc.sbuf_tensor([P, KC2, D], F16))
        mbdt_sb = ctx.enter_context(nc.sbuf_tensor([P, K1P, S], F16))
        bias_sb = ctx.enter_context(nc.sbuf_tensor([1, D], F16))
        ones_sb = ctx.enter_context(nc.sbuf_tensor([1, S], F16))
        msc_sb = ctx.enter_context(nc.sbuf_tensor([S, D], F16))
        msT_sb = ctx.enter_context(nc.sbuf_tensor([P, KC2, S], F16))
        out_sb = ctx.enter_context(nc.sbuf_tensor([P, 512], F32))
        pm_banks = [
            ctx.enter_context(nc.psum_tensor(f"pm{n}", [P, 512], F32))
            for n in range(NT)
        ]
        po_bank = ctx.enter_context(nc.psum_tensor([P, 512], F32))
        fsems = [ctx.enter_context(nc.semaphore(f"fs{g}")) for g in range(NF)]
        wasems = [ctx.enter_context(nc.semaphore(f"wa{g}")) for g in range(len(WA))]
        wbsems = [ctx.enter_context(nc.semaphore(f"wb{g}")) for g in range(len(WB))]
        csem = ctx.enter_context(nc.semaphore("csem"))
        osem = ctx.enter_context(nc.semaphore("osem"))
        s1_sem = ctx.enter_context(nc.semaphore("s1_sem"))
        cp_sem = ctx.enter_context(nc.semaphore("cp_sem"))
        tr_sem = ctx.enter_context(nc.semaphore("tr_sem"))
        s2_sem = ctx.enter_context(nc.semaphore("s2_sem"))
        act_sem = ctx.enter_context(nc.semaphore("act_sem"))
        osem2 = ctx.enter_context(nc.semaphore("osem2"))
        block = ctx.enter_context(nc.Block(no_gpsimd_drain=True))

        @block.sync
        def _(sync):
            for g in range(NF):
                st, ln = FST[g], FCH[g]
                sync.dma_start(
                    feat_sb[:, st : st + ln, :],
                    feat[st * P : (st + ln) * P, :].rearrange(
                        "(c p) d -> p c d", p=P
                    ),
                ).then_inc(fsems[g], 16)
            for g, (st, ln) in enumerate(WA):
                sync.dma_start(
                    w_sb[:, st : st + ln, :],
                    w[st * P : (st + ln) * P, :].rearrange("(c p) d -> p c d", p=P),
                ).then_inc(wasems[g], 16)
            sync.wait_ge(act_sem, 1)
            for n in (0, 2):
                sync.dma_start(
                    out[:, n * 512 : (n + 1) * 512], out_sb[n * S : (n + 1) * S, :]
                ).then_inc(osem2, 16)
            sync.wait_ge(osem2, 32)

        @block.scalar
        def _(scalar):
            scalar.dma_start(
                mbdt_sb[:], mbdt[:].rearrange("p (k j) -> p k j", k=K1P)
            ).then_inc(csem, 16)
            scalar.dma_start(bias_sb[:], bias[:]).then_inc(csem, 16)
            scalar.dma_start(ones_sb[:], onesd[:]).then_inc(csem, 16)
            for g, (st, ln) in enumerate(WB):
                scalar.dma_start(
                    w_sb[:, st : st + ln, :],
                    w[st * P : (st + ln) * P, :].rearrange("(c p) d -> p c d", p=P),
                ).then_inc(wbsems[g], 16)
            scalar.wait_ge(cp_sem, NT)
            scalar.dma_start(msT_sb[:, :, :], msc_sb[:, :], transpose=True).then_inc(
                tr_sem, 16
            )
            scalar.wait_ge(s2_sem, 1)
            nc.scalar.activation(
                out_sb[:], po_bank[:], mybir.ActivationFunctionType.Tanh
            ).then_inc(act_sem, 1)
            scalar.wait_ge(act_sem, 1)
            for n in (1, 3):
                scalar.dma_start(
                    out[:, n * 512 : (n + 1) * 512], out_sb[n * S : (n + 1) * S, :]
                ).then_inc(osem, 16)
            scalar.wait_ge(osem, 32)

        @block.vector
        def _(vector):
            vector.wait_ge(s1_sem, 1)
            last = None
            for n in range(NT):
                last = nc.vector.tensor_copy(
                    msc_sb[:, n * 512 : (n + 1) * 512], pm_banks[n][0:S, :]
                )
            last.then_inc(cp_sem, NT)

        @block.tensor
        def _(tensor):
            tensor.wait_ge(csem, 48)
            for n in range(NT):
                nc.tensor.matmul(
                    po_bank[n * S : (n + 1) * S, :],
                    ones_sb[:],
                    bias_sb[:, n * 512 : (n + 1) * 512],
                    start=True,
                    stop=False,
                    tile_position=(0, n * S),
                    skip_group_check=True,
                )
            last = None
            for g in range(NF):
                tensor.wait_ge(fsems[g], 16)
                for c in range(FCH[g]):
                    k = FST[g] + c
                    for n in range(NT):
                        last = nc.tensor.matmul(
                            pm_banks[n][0:S, :],
                            mbdt_sb[:, k, :],
                            feat_sb[:, k, n * 512 : (n + 1) * 512],
                            start=(k == 0),
                            stop=(k == K1P - 1),
                            skip_group_check=True,
                        )
            last.then_inc(s1_sem, 1)
            tensor.wait_ge(tr_sem, 16)
            # scalar-ring half (k2 8-15) lands first; consume it first
            order = []
            for g, (st, ln) in enumerate(WB):
                order.append((wbsems[g], st, ln))
            for g, (st, ln) in enumerate(WA):
                order.append((wasems[g], st, ln))
            n_mm = sum(ln for _, _, ln in order) * NT
            done = 0
            for sem, st, ln in order:
                tensor.wait_ge(sem, 16)
                for c in range(ln):
                    k2 = st + c
                    for n in range(NT):
                        done += 1
                        mm = nc.tensor.matmul(
                            po_bank[n * S : (n + 1) * S, :],
                            msT_sb[:, k2, :],
                            w_sb[:, k2, n * 512 : (n + 1) * 512],
                            start=False,
                            stop=(done == n_mm),
                            tile_position=(0, n * S),
                            skip_group_check=True,
                        )
                        if done == n_mm:
                            mm.then_inc(s2_sem, 1)

    nc.compile()
    return nc


def build_nc_mesh(warm1: int = 40, warm2: int = 24):
    """W column-sharded (1 MB/core) + SPMD mesh all-gather of the masked sums
    via remote_dma (SBUF->SBUF, XOR peers). Packed feat as in build_nc_packed.
    warm1/warm2: dummy matmuls keeping the PE HAM-warm before stage 1 / during
    the mesh wait."""
    FCH = [3, 2]
    FST = [0, 3]
    NF = len(FCH)
    nc = bacc.Bacc(
        "TRN2", target_bir_lowering=False, debug=False, num_devices=NCORES
    )

    feat = nc.dram_tensor("feat", [K1P * P, D], F16, kind="ExternalInput")
    mbdt = nc.dram_tensor("mbdt", [P, K1P * S], F16, kind="ExternalInput")
    wsh = nc.dram_tensor("wsh", [D, NS], F16, kind="ExternalInput")
    bias = nc.dram_tensor("bias", [1, NS], F16, kind="ExternalInput")
    onesd = nc.dram_tensor("ones", [1, P], F16, kind="ExternalInput")
    out = nc.dram_tensor("out", [B, NS], F32, kind="ExternalOutput")

    with ExitStack() as ctx:
        feat_sb = ctx.enter_context(nc.sbuf_tensor([P, K1P, D], F16))
        wsh_sb = ctx.enter_context(nc.sbuf_tensor([P, KC2, NS], F16))
        mbdt_sb = ctx.enter_context(nc.sbuf_tensor([P, K1P, S], F16))
        bias_sb = ctx.enter_context(nc.sbuf_tensor([1, NS], F16))
        ones_sb = ctx.enter_context(nc.sbuf_tensor([1, P], F16))
        msc_sb = ctx.enter_context(nc.sbuf_tensor([S, D], F16))
        msT_sb = ctx.enter_context(nc.sbuf_tensor([P, KC2 * S], F16))
        msTall_sb = ctx.enter_context(nc.sbuf_tensor([P, NCORES, KC2, S], F16))
        out_sb = ctx.enter_context(nc.sbuf_tensor([P, 2 * NS], F32))
        pm_banks = [
            ctx.enter_context(nc.psum_tensor(f"pm{n}", [P, 512], F32))
            for n in range(NT)
        ]
        po_bank = ctx.enter_context(nc.psum_tensor([P, 512], F32))
        pw_bank = ctx.enter_context(nc.psum_tensor([P, S], F32))
        fsems = [ctx.enter_context(nc.semaphore(f"fs{g}")) for g in range(NF)]
        csem = ctx.enter_context(nc.semaphore("csem"))
        wsem = ctx.enter_context(nc.semaphore("wsem"))
        s1_sem = ctx.enter_context(nc.semaphore("s1_sem"))
        cp_sem = ctx.enter_context(nc.semaphore("cp_sem"))
        tr_sem = ctx.enter_context(nc.semaphore("tr_sem"))
        lsem = ctx.enter_context(nc.semaphore("lsem"))
        rsem = ctx.enter_context(nc.semaphore("rsem"))
        s2_sems = [ctx.enter_context(nc.semaphore(f"s2g{g}")) for g in range(2)]
        act_sems = [ctx.enter_context(nc.semaphore(f"act{g}")) for g in range(2)]
        osem = ctx.enter_context(nc.semaphore("osem"))
        block = ctx.enter_context(nc.Block(no_gpsimd_drain=True))

        @block.sync
        def _(sync):
            for g in range(NF):
                st, ln = FST[g], FCH[g]
                sync.dma_start(
                    feat_sb[:, st : st + ln, :],
                    feat[st * P : (st + ln) * P, :].rearrange(
                        "(c p) d -> p c d", p=P
                    ),
                ).then_inc(fsems[g], 16)
            for grp in range(2):
                sync.wait_ge(act_sems[grp], 1)
                sync.dma_start(
                    out[grp * P : (grp + 1) * P, :],
                    out_sb[:, grp * NS : (grp + 1) * NS],
                ).then_inc(osem, 16)
            sync.wait_ge(osem, 32)

        @block.scalar
        def _(scalar):
            scalar.dma_start(
                mbdt_sb[:], mbdt[:].rearrange("p (k j) -> p k j", k=K1P)
            ).then_inc(csem, 16)
            scalar.dma_start(bias_sb[:], bias[:]).then_inc(csem, 16)
            scalar.dma_start(ones_sb[:], onesd[:]).then_inc(csem, 16)
            scalar.dma_start(
                wsh_sb[:], wsh[:, :].rearrange("(k p) n -> p k n", p=P)
            ).then_inc(wsem, 16)
            scalar.wait_ge(cp_sem, NT)
            scalar.dma_start(
                msT_sb[:].rearrange("p (k j) -> p k j", k=KC2),
                msc_sb[:, :],
                transpose=True,
            ).then_inc(tr_sem, 16)
            for grp in range(2):
                scalar.wait_ge(s2_sems[grp], 1)
                nc.scalar.activation(
                    out_sb[:, grp * NS : (grp + 1) * NS],
                    po_bank[:, grp * NS : (grp + 1) * NS],
                    mybir.ActivationFunctionType.Tanh,
                ).then_inc(act_sems[grp], 1)

        @block.vector
        def _(vector):
            vector.wait_ge(s1_sem, 1)
            last = None
            for n in range(NT):
                last = nc.vector.tensor_copy(
                    msc_sb[:, n * 512 : (n + 1) * 512], pm_banks[n][0:S, :]
                )
            last.then_inc(cp_sem, NT)

        @block.gpsimd
        def _(gp):
            gp.wait_ge(tr_sem, 16)
            # XOR-relative broadcasts: broadcast j lands my msT in the slot j
            # of core (my ^ j). Slot j on this core therefore holds the msT of
            # core (my ^ j); the host un-permutes output rows accordingly.
            for j in range(NCORES):
                rdests = [None] * NCORES
                rdests[j] = (0, j)
                gp.remote_dma_broadcast(
                    out_ap=msTall_sb[:, j, :, :],
                    in_ap=msT_sb[:, :],
                    remote_sem=rsem,
                    local_sem=lsem,
                    rdests=rdests,
                )
            gp.trigger_dma()
            gp.wait_ge(lsem, 8 * 16)

        @block.tensor
        def _(tensor):
            tensor.wait_ge(csem, 48)
            for grp in range(2):
                nc.tensor.matmul(
                    po_bank[:, grp * NS : (grp + 1) * NS],
                    ones_sb[:],
                    bias_sb[:],
                    start=True,
                    stop=False,
                    skip_group_check=True,
                )
            # HAM warm-up so stage 1 runs at 2.4 GHz
            for _ in range(warm1):
                nc.tensor.matmul(
                    pw_bank[0:S, :],
                    mbdt_sb[:, 0, :],
                    mbdt_sb[:, 0, :],
                    start=True,
                    stop=True,
                    skip_group_check=True,
                )
            last = None
            for g in range(NF):
                tensor.wait_ge(fsems[g], 16)
                for c in range(FCH[g]):
                    k = FST[g] + c
                    for n in range(NT):
                        last = nc.tensor.matmul(
                            pm_banks[n][0:S, :],
                            mbdt_sb[:, k, :],
                            feat_sb[:, k, n * 512 : (n + 1) * 512],
                            start=(k == 0),
                            stop=(k == K1P - 1),
                            skip_group_check=True,
                        )
            last.then_inc(s1_sem, 1)
            # stay warm across the mesh exchange
            for _ in range(warm2):
                nc.tensor.matmul(
                    pw_bank[0:S, :],
                    mbdt_sb[:, 0, :],
                    mbdt_sb[:, 0, :],
                    start=True,
                    stop=True,
                    skip_group_check=True,
                )
            tensor.wait_ge(wsem, 16)
            tensor.wait_ge(rsem, 16)  # 8 arrivals x (16/8)=2 each
            mT = msTall_sb[:, :, :, :]
            for grp in range(2):
                lastb = None
                for k2 in range(KC2):
                    lastb = nc.tensor.matmul(
                        po_bank[:, grp * NS : (grp + 1) * NS],
                        mT[:, 4 * grp : 4 * grp + 4, k2, :],
                        wsh_sb[:, k2, :],
                        start=False,
                        stop=(k2 == KC2 - 1),
                        skip_group_check=True,
                    )
                lastb.then_inc(s2_sems[grp], 1)

    nc.compile()
    return nc


def _host_prep_mesh(inputs: dict):
    feat = np.asarray(inputs["attribute_feat"], dtype=np.float32)
    label = np.asarray(inputs["attribute_label"])
    w16 = np.asarray(inputs["W"], dtype=np.float32).astype(np.float16)
    b = np.asarray(inputs["b"], dtype=np.float32).astype(np.float16).reshape(1, D)
    mask = np.asarray(label) > 0

    in_maps = []
    for c in range(NCORES):
        m_c = mask[c * S : (c + 1) * S]
        f_c = feat[c * S : (c + 1) * S]
        smp_idx, att_idx = np.nonzero(m_c)
        nrows = len(smp_idx)
        if nrows > K1P * P:
            return None
        packed = np.zeros((K1P * P, D), np.float16)
        packed[:nrows] = f_c[smp_idx, att_idx].astype(np.float16)
        sel = np.zeros((K1P * P, S), np.float32)
        sel[np.arange(nrows), smp_idx] = 1.0
        sel_dev = np.ascontiguousarray(
            sel.reshape(K1P, P, S).transpose(1, 0, 2)
        ).reshape(P, K1P * S)
        in_maps.append(
            {
                "feat": packed,
                "mbdt": sel_dev.astype(np.float16),
                "wsh": np.ascontiguousarray(w16[:, c * NS : (c + 1) * NS]),
                "bias": np.ascontiguousarray(b[:, c * NS : (c + 1) * NS]),
                "ones": np.ones((1, P), np.float16),
            }
        )
    return in_maps


def build_nc_dp8():
    """Fallback: full unpacked feat (KC1=8), DVE 32x32 transposes."""
    cdt = F16
    cf = 4
    WCH = [4, 4, 4, 4]
    WST = [0, 4, 8, 12]
    NF, NW = KC1 // cf, len(WCH)
    nc = bacc.Bacc("TRN2", target_bir_lowering=False, debug=False)

    feat = nc.dram_tensor("feat", [S * A, D], cdt, kind="ExternalInput")
    mbdt = nc.dram_tensor("mbdt", [P, KC1 * S], cdt, kind="ExternalInput")
    w = nc.dram_tensor("w", [D, D], cdt, kind="ExternalInput")
    bias = nc.dram_tensor("bias", [1, D], cdt, kind="ExternalInput")
    onesd = nc.dram_tensor("ones", [1, S], cdt, kind="ExternalInput")
    out = nc.dram_tensor("out", [S, D], F32, kind="ExternalOutput")

    with ExitStack() as ctx:
        feat_sb = ctx.enter_context(nc.sbuf_tensor([P, KC1, D], cdt))
        w_sb = ctx.enter_context(nc.sbuf_tensor([P, KC2, D], cdt))
        mbdt_sb = ctx.enter_context(nc.sbuf_tensor([P, KC1, S], cdt))
        bias_sb = ctx.enter_context(nc.sbuf_tensor([1, D], cdt))
        ones_sb = ctx.enter_context(nc.sbuf_tensor([1, S], cdt))
        msc_sb = ctx.enter_context(nc.sbuf_tensor([P, 512], cdt))
        msT_sb = ctx.enter_context(nc.sbuf_tensor([P, KC2, S], cdt))
        out_sb = ctx.enter_context(nc.sbuf_tensor([P, 512], F32))
        pm_bank = ctx.enter_context(nc.psum_tensor([P, 512], F32))
        po_bank = ctx.enter_context(nc.psum_tensor([P, 512], F32))
        fsems = [ctx.enter_context(nc.semaphore(f"fs{g}")) for g in range(NF)]
        wsems = [ctx.enter_context(nc.semaphore(f"ws{g}")) for g in range(NW)]
        csem = ctx.enter_context(nc.semaphore("csem"))
        osem = ctx.enter_context(nc.semaphore("osem"))
        s1_sem = ctx.enter_context(nc.semaphore("s1_sem"))
        tr_sem = ctx.enter_context(nc.semaphore("tr_sem"))
        s2_sem = ctx.enter_context(nc.semaphore("s2_sem"))
        act_sem = ctx.enter_context(nc.semaphore("act_sem"))
        osem2 = ctx.enter_context(nc.semaphore("osem2"))
        block = ctx.enter_context(nc.Block(no_gpsimd_drain=True))

        @block.sync
        def _(sync):
            for g in range(NF):
                sync.dma_start(
                    feat_sb[:, g * cf : (g + 1) * cf, :],
                    feat[g * cf * P : (g + 1) * cf * P, :].rearrange(
                        "(c p) d -> p c d", p=P
                    ),
                ).then_inc(fsems[g], 16)
            for g in range(NW):
                st, ln = WST[g], WCH[g]
                sync.dma_start(
                    w_sb[:, st : st + ln, :],
                    w[st * P : (st + ln) * P, :].rearrange("(c p) d -> p c d", p=P),
                ).then_inc(wsems[g], 16)
            sync.wait_ge(act_sem, 1)
            for n in (0, 2):
                sync.dma_start(
                    out[:, n * 512 : (n + 1) * 512], out_sb[n * S : (n + 1) * S, :]
                ).then_inc(osem2, 16)
            sync.wait_ge(osem2, 32)

        @block.scalar
        def _(scalar):
            scalar.dma_start(
                mbdt_sb[:], mbdt[:].rearrange("p (k j) -> p k j", k=KC1)
            ).then_inc(csem, 16)
            scalar.dma_start(bias_sb[:], bias[:]).then_inc(csem, 16)
            scalar.dma_start(ones_sb[:], onesd[:]).then_inc(csem, 16)
            scalar.wait_ge(s2_sem, 1)
            nc.scalar.activation(
                out_sb[:], po_bank[:], mybir.ActivationFunctionType.Tanh
            ).then_inc(act_sem, 1)
            scalar.wait_ge(act_sem, 1)
            for n in (1, 3):
                scalar.dma_start(
                    out[:, n * 512 : (n + 1) * 512], out_sb[n * S : (n + 1) * S, :]
                ).then_inc(osem, 16)
            scalar.wait_ge(osem, 32)

        @block.vector
        def _(vector):
            vector.wait_ge(s1_sem, 1)
            nc.vector.tensor_copy(msc_sb[:], pm_bank[:])
            nc.vector.drain()
            last = None
            for n in range(NT):
                for q in range(512 // 32):
                    d0 = n * 512 + q * 32
                    k2, r = divmod(d0, P)
                    last = nc.vector.transpose(
                        msT_sb[r : r + 32, k2, :],
                        msc_sb[n * S : (n + 1) * S, q * 32 : (q + 1) * 32],
                    )
            last.then_inc(tr_sem, 1)

        @block.tensor
        def _(tensor):
            tensor.wait_ge(csem, 48)
            for n in range(NT):
                nc.tensor.matmul(
                    po_bank[n * S : (n + 1) * S, :],
                    ones_sb[:],
                    bias_sb[:, n * 512 : (n + 1) * 512],
                    start=True,
                    stop=False,
                    tile_position=(0, n * S),
                    skip_group_check=True,
                )
            last = None
            for g in range(NF):
                tensor.wait_ge(fsems[g], 16)
                for c in range(cf):
                    k = g * cf + c
                    for n in range(NT):
                        last = nc.tensor.matmul(
                            pm_bank[n * S : (n + 1) * S, :],
                            mbdt_sb[:, k, :],
                            feat_sb[:, k, n * 512 : (n + 1) * 512],
                            start=(k == 0),
                            stop=(k == KC1 - 1),
                            tile_position=(0, n * S),
                            skip_group_check=True,
                        )
            last.then_inc(s1_sem, 1)
            tensor.wait_ge(tr_sem, 1)
            lastb = None
            for g in range(NW):
                tensor.wait_ge(wsems[g], 16)
                for c in range(WCH[g]):
                    k2 = WST[g] + c
                    for n in range(NT):
                        lastb = nc.tensor.matmul(
                            po_bank[n * S : (n + 1) * S, :],
                            msT_sb[:, k2, :],
                            w_sb[:, k2, n * 512 : (n + 1) * 512],
                            start=False,
                            stop=(k2 == KC2 - 1),
                            tile_position=(0, n * S),
                            skip_group_check=True,
                        )
            lastb.then_inc(s2_sem, 1)

    nc.compile()
    return nc


def _mbd_blockdiag_unpacked(mask_c: np.ndarray) -> np.ndarray:
    mbd = np.zeros((KC1, P, S), np.float32)
    for k in range(KC1):
        for sl in range(P // A):
            smp = (P // A) * k + sl
            mbd[k, sl * A : (sl + 1) * A, smp] = mask_c[smp]
    return np.ascontiguousarray(mbd.transpose(1, 0, 2)).reshape(P, KC1 * S)


def _host_prep_packed(inputs: dict):
    feat = np.asarray(inputs["attribute_feat"], dtype=np.float32)
    label = np.asarray(inputs["attribute_label"])
    w16 = np.asarray(inputs["W"], dtype=np.float32).astype(np.float16)
    b = np.asarray(inputs["b"], dtype=np.float32).astype(np.float16).reshape(1, D)
    mask = np.asarray(label) > 0

    in_maps = []
    for c in range(NCORES):
        m_c = mask[c * S : (c + 1) * S]  # [S, A] bool
        f_c = feat[c * S : (c + 1) * S]  # [S, A, D]
        smp_idx, att_idx = np.nonzero(m_c)
        nrows = len(smp_idx)
        if nrows > K1P * P:
            return None  # overflow: caller falls back to dp8
        packed = np.zeros((K1P * P, D), np.float16)
        packed[:nrows] = f_c[smp_idx, att_idx].astype(np.float16)
        sel = np.zeros((K1P * P, S), np.float32)
        sel[np.arange(nrows), smp_idx] = 1.0
        # device layout [P, (k, s)]
        sel_dev = np.ascontiguousarray(
            sel.reshape(K1P, P, S).transpose(1, 0, 2)
        ).reshape(P, K1P * S)
        in_maps.append(
            {
                "feat": packed,
                "mbdt": sel_dev.astype(np.float16),
                "w": w16,
                "bias": b,
                "ones": np.ones((1, S), np.float16),
            }
        )
    return in_maps


def _host_prep_dp8(inputs: dict):
    feat = np.asarray(inputs["attribute_feat"], dtype=np.float32)
    label = np.asarray(inputs["attribute_label"])
    w16 = np.asarray(inputs["W"], dtype=np.float32).astype(np.float16)
    b = np.asarray(inputs["b"], dtype=np.float32).astype(np.float16).reshape(1, D)
    mask = (np.asarray(label) > 0).astype(np.float32)
    in_maps = []
    for c in range(NCORES):
        in_maps.append(
            {
                "feat": feat[c * S : (c + 1) * S]
                .reshape(S * A, D)
                .astype(np.float16),
                "mbdt": _mbd_blockdiag_unpacked(mask[c * S : (c + 1) * S]).astype(
                    np.float16
                ),
                "w": w16,
                "bias": b,
                "ones": np.ones((1, S), np.float16),
            }
        )
    return in_maps


_NC_CACHE: dict = {}


def run(inputs: dict, compute_dtype: str = "fp16", trace: bool = False):
    from concourse.bass_utils import run_bass_kernel_spmd

    impl = os.environ.get("GNN_KERNEL_IMPL", "packed4")
    in_maps = None
    if impl == "mesh":
        in_maps = _host_prep_mesh(inputs)
        if in_maps is None:
            impl = "dp8"
    if impl in ("packed", "packed2", "packed3", "packed4", "packed5"):
        in_maps = _host_prep_packed(inputs)
        if in_maps is None:
            impl = "dp8"
    if impl == "dp8":
        in_maps = _host_prep_dp8(inputs)
    if impl not in _NC_CACHE:
        builders = {
            "packed": build_nc_packed,
            "packed4": lambda: build_nc_packed(out16=True),
            "packed5": build_nc_packed5,
            "packed2": build_nc_packed2,
            "packed3": build_nc_packed3,
            "mesh": build_nc_mesh,
            "dp8": build_nc_dp8,
        }
        _NC_CACHE[impl] = builders[impl]()
    nc = _NC_CACHE[impl]
    res = run_bass_kernel_spmd(nc, in_maps, list(range(NCORES)), trace=trace)
    if impl == "mesh":
        out = np.empty((B, D), np.float32)
        for c in range(NCORES):
            oc = np.asarray(res.results[c]["out"], dtype=np.float32)  # [B, NS]
            for j in range(NCORES):
                b0 = S * (c ^ j)
                out[b0 : b0 + S, c * NS : (c + 1) * NS] = oc[S * j : S * j + S]
    elif impl in ("packed3", "packed4", "packed5"):
        # outb[g*S+s, u] = out[s, g*512+u]; un-block per core then stack
        parts = []
        for c in range(NCORES):
            ob = np.asarray(res.results[c]["outb"], dtype=np.float32)  # [128, 512]
            parts.append(
                ob.reshape(NT, S, 512).transpose(1, 0, 2).reshape(S, D)
            )
        out = np.concatenate(parts, axis=0)
    else:
        out = np.concatenate(
            [res.results[c]["out"] for c in range(NCORES)], axis=0
        ).astype(np.float32)
    return out, res


def kernel(**inputs) -> np.ndarray:
    out, _ = run(inputs)
    return out
